# revision 6
# baseline (speedup 1.0000x reference)
# GCN layer kernel for Trainium2: out[b] = relu((a[b] @ x[b]) @ W) * mask[b]
#
# Sharding: data-parallel over the batch (graph) dim. B=8 graphs, 8 cores,
# one graph per core; W replicated. Inputs are the FULL tensors; shards are
# sliced host-side and the per-core outputs stacked back together.
#
# Per-core dataflow (a: [2048,2048], x: [2048,512], W: [512,512]):
#   - a must be contracted over its column index; TensorE contracts over the
#     partition (row) index of both operands, so a is transposed on-chip via
#     PE transpose (fp32 has no DMA-transpose path), 128x128 tiles.
#   - t^T[f,n] = sum_m x[m,f] * aT[m,n]:  lhsT = x (as stored), rhs = aT.
#   - out[n,d] = sum_f t^T[f,n] * W[f,d]: lhsT = t^T, rhs = W (as stored),
#     which lands out in [n,d] layout for a direct DMA store.
#   - Matmuls run as float32r (full-rate fp32 mode; fp32 proper is 4x slower).
#     walrus requires f32r matmul operands to be produced by instructions that
#     round to f32r, so every operand tile is written by a DVE/ACT copy with
#     float32r output dtype (DMA-fed x/W get a one-time rounding copy).
#     Transposes stay fp32 (their a-strip/identity inputs are not rounded);
#     the PSUM->SBUF copyback CAST does the f32r rounding.
#   - mask[n] = any(x[n,:] != 0), computed as sum(|x[n,:]|) > 0, and applied
#     fused into the ReLU: relu(mask * t) == mask * relu(t) since mask >= 0.
#
# Schedule notes (from NTFF traces):
#   - The PE HAM clock-gate only counts REGULAR matmuls as activity;
#     transpose-mode matmuls run on the gated clock but do not un-throttle
#     it. A warm-up burst of fp32 identity matmuls (overlapping the initial
#     DMA wait) plus periodic dummy matmuls inside transpose-only stretches
#     keep the PE at K=8/8 (2.4 GHz).
#   - DMA order: nj0's a-strips first, then x column-chunks, then W, so the
#     PE ramps with the DMA instead of idling behind a bulk x/W load.
#   - nj0 transposes are grouped per a-strip (j-outer) to start as soon as
#     strip 0 lands. nj>=1 (strips prefetched) run mi-outer with mm1
#     software-pipelined one m-tile behind the transposes, so regular
#     matmuls interleave and no bulk copyback wait exists.

import numpy as np

B, N, F, D = 8, 2048, 512, 512
P = 128
NT = N // P        # 16 row-tiles of n (and of m, since a is square)
FT = F // P        # 4 tiles of f
NCHUNK = 512       # n is processed in chunks of 512 columns
NJ = N // NCHUNK   # 4
NSUB = NCHUNK // P # 4

_CACHE = {}


def _build_nc():
    from contextlib import ExitStack

    from concourse import bacc, mybir, tile
    from concourse.masks import make_identity

    f32 = mybir.dt.float32
    f32r = mybir.dt.float32r
    AF = mybir.ActivationFunctionType

    nc = bacc.Bacc(None)
    a_d = nc.dram_tensor("a", [N, N], f32, kind="ExternalInput")
    x_d = nc.dram_tensor("x", [N, F], f32, kind="ExternalInput")
    w_d = nc.dram_tensor("kernel", [F, D], f32, kind="ExternalInput")
    o_d = nc.dram_tensor("out", [N, D], f32, kind="ExternalOutput")

    with tile.TileContext(nc) as tc, ExitStack() as ctx:
        const = ctx.enter_context(tc.tile_pool(name="const", bufs=1))
        xp = ctx.enter_context(tc.tile_pool(name="xp", bufs=1))
        wp = ctx.enter_context(tc.tile_pool(name="wp", bufs=1))
        a_pool = ctx.enter_context(tc.tile_pool(name="a_pool", bufs=5))
        atp = ctx.enter_context(tc.tile_pool(name="atp", bufs=2))
        ttp = ctx.enter_context(tc.tile_pool(name="ttp", bufs=2))
        outp = ctx.enter_context(tc.tile_pool(name="outp", bufs=3))
        scr = ctx.enter_context(tc.tile_pool(name="scr", bufs=2))
        # PSUM budget (8 banks): 1 transpose + 1 warm-up/dummy + 4 mm1 + 2 out
        ps_tp = ctx.enter_context(tc.tile_pool(name="ps_tp", bufs=1, space="PSUM"))
        ps_wu = ctx.enter_context(tc.tile_pool(name="ps_wu", bufs=1, space="PSUM"))
        ps_mm = ctx.enter_context(tc.tile_pool(name="ps_mm", bufs=4, space="PSUM"))
        ps_o = ctx.enter_context(tc.tile_pool(name="ps_o", bufs=2, space="PSUM"))

        ident = const.tile([P, P], f32)
        make_identity(nc, ident[:])

        def warm_mm():
            # fp32 identity matmul: registers as HAM activity, output unused
            pw = ps_wu.tile([P, P], f32, tag="warm")
            nc.tensor.matmul(pw[:], lhsT=ident[:], rhs=ident[:], start=True, stop=True)

        # HAM warm-up overlapping the initial DMA wait (>3.4us of cold-clock
        # PE activity flips the clock gate to 2.4 GHz before real work).
        for wu in range(10):
            warm_mm()

        # a-strips for nj=0 issued first so transposes can start ASAP
        first_strips = []
        for j in range(NSUB):
            ast = a_pool.tile([P, N], f32, tag="a_strip")
            nc.sync.dma_start(ast[:], a_d[j * P : (j + 1) * P, :])
            first_strips.append(ast)

        # x: DMA fp32 column-chunks into scratch, round to f32r resident tile
        # [p, 16, 512] (m on partitions). mm1's fi-th accumulation needs only
        # column-chunk fi, so mm1 can ramp while later chunks stream in.
        x_r = xp.tile([P, NT, F], f32r)
        for c in range(4):
            xl = scr.tile([P, NT, P], f32, tag="load_scr")
            nc.sync.dma_start(
                xl[:], x_d[:, c * P : (c + 1) * P].rearrange("(o p) f -> p o f", p=P)
            )
            nc.vector.tensor_copy(x_r[:, :, c * P : (c + 1) * P], xl[:])

        w_r = wp.tile([P, FT, D], f32r)
        wl = scr.tile([P, FT, D], f32, tag="load_scr")
        nc.sync.dma_start(wl[:], w_d[:].rearrange("(o p) d -> p o d", p=P))
        nc.vector.tensor_copy(w_r[:], wl[:])

        # mask[n] = (sum_f |x[n,f]|) > 0, one column per n-row-tile.
        # (|round_f32r(v)| > 0 iff |v| > 0: mantissa truncation keeps exponent.)
        sumabs = const.tile([P, NT], f32)
        mask_sb = const.tile([P, NT], f32)
        for ni in range(NT):
            abs_scr = scr.tile([P, F], f32, tag="abs_scr")
            nc.scalar.activation(
                abs_scr[:], x_r[:, ni], AF.Abs, accum_out=sumabs[:, ni : ni + 1]
            )
        nc.vector.tensor_scalar(
            mask_sb[:], sumabs[:], 0.0, None, mybir.AluOpType.is_gt
        )

        cb = 0  # copyback counter for DVE/ACT alternation (5/8 to DVE)

        def copyback(dst, src):
            nonlocal cb
            if cb % 8 < 5:
                nc.vector.tensor_copy(dst, src)
            else:
                nc.scalar.copy(dst, src)
            cb += 1

        for nj in range(NJ):
            if nj == 0:
                a_strips = first_strips
            else:
                a_strips = []
                for j in range(NSUB):
                    ast = a_pool.tile([P, N], f32, tag="a_strip")
                    ni = nj * NSUB + j
                    nc.sync.dma_start(ast[:], a_d[ni * P : (ni + 1) * P, :])
                    a_strips.append(ast)

            at_sb = atp.tile([P, NT, NCHUNK], f32r, tag="at")
            tt_sb = ttp.tile([P, FT, NCHUNK], f32r, tag="tt")

            if nj == 0:
                # j-outer: transposes track the strip DMAs. Dummy matmuls
                # every other quad keep the HAM clock-gate open (transposes
                # don't count as activity).
                for j in range(NSUB):
                    src = a_strips[j][:]
                    for q in range(NT // 4):
                        ps = ps_tp.tile([P, NCHUNK], f32, tag="pst")
                        for k in range(4):
                            mi = q * 4 + k
                            nc.tensor.transpose(
                                ps[:, k * P : (k + 1) * P],
                                src[:, mi * P : (mi + 1) * P],
                                ident[:],
                            )
                        dst = at_sb[:, q * 4 : (q + 1) * 4, j * P : (j + 1) * P]
                        copyback(dst, ps[:].rearrange("p (q f) -> p q f", q=4))
                        if q % 2 == 1:
                            warm_mm()
                # mm1, fi-outer: accumulation fi needs x column-chunk fi only
                for fi in range(FT):
                    pt = ps_mm.tile([P, NCHUNK], f32, tag="psm")
                    for mi in range(NT):
                        nc.tensor.matmul(
                            pt[:],
                            lhsT=x_r[:, mi, fi * P : (fi + 1) * P],
                            rhs=at_sb[:, mi],
                            start=(mi == 0),
                            stop=(mi == NT - 1),
                        )
                    copyback(tt_sb[:, fi], pt[:])
            else:
                # mi-outer with mm1 pipelined one m-tile behind the
                # transposes: regular matmuls interleave with transposes, so
                # the HAM stays warm and copybacks hide behind PE work.
                pt = [
                    ps_mm.tile([P, NCHUNK], f32, tag="psm", name=f"pt_{nj}_{fi}")
                    for fi in range(FT)
                ]

                def mm1_step(mi):
                    for fi in range(FT):
                        nc.tensor.matmul(
                            pt[fi][:],
                            lhsT=x_r[:, mi, fi * P : (fi + 1) * P],
                            rhs=at_sb[:, mi],
                            start=(mi == 0),
                            stop=(mi == NT - 1),
                        )

                for mi in range(NT):
                    ps = ps_tp.tile([P, NCHUNK], f32, tag="pst")
                    for j in range(NSUB):
                        nc.tensor.transpose(
                            ps[:, j * P : (j + 1) * P],
                            a_strips[j][:, mi * P : (mi + 1) * P],
                            ident[:],
                        )
                    copyback(at_sb[:, mi], ps[:])
                    if mi >= 1:
                        mm1_step(mi - 1)
                mm1_step(NT - 1)
                for fi in range(FT):
                    copyback(tt_sb[:, fi], pt[fi][:])

            # out rows for this chunk: accumulate over the 4 f-tiles, then
            # fused relu+mask on ACT, then store.
            for ns in range(NSUB):
                po = ps_o.tile([P, D], f32, tag="pso")
                for fi in range(FT):
                    nc.tensor.matmul(
                        po[:],
                        lhsT=tt_sb[:, fi, ns * P : (ns + 1) * P],
                        rhs=w_r[:, fi],
                        start=(fi == 0),
                        stop=(fi == FT - 1),
                    )
                ni = nj * NSUB + ns
                ob = outp.tile([P, D], f32, tag="ob")
                nc.scalar.activation(
                    ob[:], po[:], AF.Relu, scale=mask_sb[:, ni : ni + 1]
                )
                nc.sync.dma_start(o_d[ni * P : (ni + 1) * P, :], ob[:])

    nc.compile()
    return nc


def get_nc():
    if "nc" not in _CACHE:
        _CACHE["nc"] = _build_nc()
    return _CACHE["nc"]


def kernel(**inputs) -> np.ndarray:
    from concourse.bass_utils import run_bass_kernel_spmd

    x = np.ascontiguousarray(np.asarray(inputs["x"], dtype=np.float32))
    a = np.ascontiguousarray(np.asarray(inputs["a"], dtype=np.float32))
    w = np.ascontiguousarray(np.asarray(inputs["kernel"], dtype=np.float32))
    assert x.shape == (B, N, F) and a.shape == (B, N, N) and w.shape == (F, D)

    nc = get_nc()
    in_maps = [{"a": a[b], "x": x[b], "kernel": w} for b in range(B)]
    res = run_bass_kernel_spmd(nc, in_maps, core_ids=list(range(B)))
    return np.stack([res.results[b]["out"] for b in range(B)], axis=0)


# revision 7
# speedup vs baseline: 1.0769x; 1.0769x over previous
# GCN layer kernel for Trainium2: out[b] = relu((a[b] @ x[b]) @ W) * mask[b]
#
# Sharding: data-parallel over the batch (graph) dim. B=8 graphs, 8 cores,
# one graph per core; W replicated. Inputs are the FULL tensors; shards are
# sliced host-side and the per-core outputs stacked back together.
#
# Per-core dataflow (a: [2048,2048], x: [2048,512], W: [512,512]):
#   - a must be contracted over its column index; TensorE contracts over the
#     partition (row) index of both operands, so a is transposed on-chip via
#     PE transpose (fp32 has no DMA-transpose path), 128x128 tiles.
#   - t^T[f,n] = sum_m x[m,f] * aT[m,n]:  lhsT = x (as stored), rhs = aT.
#   - out[n,d] = sum_f t^T[f,n] * W[f,d]: lhsT = t^T, rhs = W (as stored),
#     which lands out in [n,d] layout for a direct DMA store.
#   - Matmuls run as float32r (full-rate fp32 mode; fp32 proper is 4x slower).
#     walrus requires f32r matmul operands to be produced by instructions that
#     round to f32r, so every operand tile is written by a DVE/ACT copy with
#     float32r output dtype (DMA-fed x/W get a one-time rounding copy).
#     Transposes stay fp32 (their a-strip/identity inputs are not rounded);
#     the PSUM->SBUF copyback CAST does the f32r rounding.
#   - mask[n] = any(x[n,:] != 0), computed as sum(|x[n,:]|) > 0, and applied
#     fused into the ReLU: relu(mask * t) == mask * relu(t) since mask >= 0.
#
# Schedule notes (from NTFF traces):
#   - The PE HAM clock-gate only counts REGULAR matmuls as activity;
#     transpose-mode matmuls run on the gated clock but do not un-throttle
#     it. A warm-up burst of fp32 identity matmuls (overlapping the initial
#     DMA wait) plus dummy matmuls inside transpose-only stretches and at
#     chunk boundaries keep the PE at K=8/8 (2.4 GHz).
#   - DMA order: nj0's a-strips first, then x column-chunks, then W, so the
#     PE ramps with the DMA instead of idling behind a bulk x/W load.
#   - nj0 transposes are grouped per a-strip (j-outer) to start as soon as
#     strip 0 lands. nj>=1 (strips prefetched) run mi-outer with mm1
#     software-pipelined one m-tile behind the transposes, so regular
#     matmuls interleave with transposes.
#   - The 16 mask |x| reductions are spread through nj0's mm1 phase so they
#     don't clog ACT ahead of the transpose copybacks.
#   - PSUM: 2 transpose + 4 mm1 + 2 out banks; warm-up/dummy matmuls borrow
#     the out-pool slots (idle at those points).

import numpy as np

B, N, F, D = 8, 2048, 512, 512
P = 128
NT = N // P        # 16 row-tiles of n (and of m, since a is square)
FT = F // P        # 4 tiles of f
NCHUNK = 512       # n is processed in chunks of 512 columns
NJ = N // NCHUNK   # 4
NSUB = NCHUNK // P # 4

_CACHE = {}


def _build_nc():
    from contextlib import ExitStack

    from concourse import bacc, mybir, tile
    from concourse.masks import make_identity

    f32 = mybir.dt.float32
    f32r = mybir.dt.float32r
    AF = mybir.ActivationFunctionType

    nc = bacc.Bacc(None)
    a_d = nc.dram_tensor("a", [N, N], f32, kind="ExternalInput")
    x_d = nc.dram_tensor("x", [N, F], f32, kind="ExternalInput")
    w_d = nc.dram_tensor("kernel", [F, D], f32, kind="ExternalInput")
    o_d = nc.dram_tensor("out", [N, D], f32, kind="ExternalOutput")

    with tile.TileContext(nc) as tc, ExitStack() as ctx:
        const = ctx.enter_context(tc.tile_pool(name="const", bufs=1))
        xp = ctx.enter_context(tc.tile_pool(name="xp", bufs=1))
        wp = ctx.enter_context(tc.tile_pool(name="wp", bufs=1))
        a_pool = ctx.enter_context(tc.tile_pool(name="a_pool", bufs=5))
        atp = ctx.enter_context(tc.tile_pool(name="atp", bufs=2))
        ttp = ctx.enter_context(tc.tile_pool(name="ttp", bufs=2))
        outp = ctx.enter_context(tc.tile_pool(name="outp", bufs=3))
        scr = ctx.enter_context(tc.tile_pool(name="scr", bufs=2))
        ps_tp = ctx.enter_context(tc.tile_pool(name="ps_tp", bufs=2, space="PSUM"))
        ps_mm = ctx.enter_context(tc.tile_pool(name="ps_mm", bufs=4, space="PSUM"))
        ps_o = ctx.enter_context(tc.tile_pool(name="ps_o", bufs=2, space="PSUM"))

        ident = const.tile([P, P], f32)
        make_identity(nc, ident[:])

        def warm_mm():
            # fp32 identity matmul: registers as HAM activity, output unused.
            # Borrows an out-pool PSUM slot (idle during transpose stretches).
            pw = ps_o.tile([P, D], f32, tag="pso", name="pw")
            nc.tensor.matmul(
                pw[:, :P], lhsT=ident[:], rhs=ident[:], start=True, stop=True
            )

        # HAM warm-up overlapping the initial DMA wait (>3.4us of cold-clock
        # PE activity flips the clock gate to 2.4 GHz before real work).
        for wu in range(10):
            warm_mm()

        # a-strips for nj=0 issued first so transposes can start ASAP
        first_strips = []
        for j in range(NSUB):
            ast = a_pool.tile([P, N], f32, tag="a_strip")
            nc.sync.dma_start(ast[:], a_d[j * P : (j + 1) * P, :])
            first_strips.append(ast)

        # x: DMA fp32 column-chunks into scratch, round to f32r resident tile
        # [p, 16, 512] (m on partitions). mm1's fi-th accumulation needs only
        # column-chunk fi, so mm1 can ramp while later chunks stream in.
        x_r = xp.tile([P, NT, F], f32r)
        for c in range(4):
            xl = scr.tile([P, NT, P], f32, tag="load_scr")
            nc.sync.dma_start(
                xl[:], x_d[:, c * P : (c + 1) * P].rearrange("(o p) f -> p o f", p=P)
            )
            nc.vector.tensor_copy(x_r[:, :, c * P : (c + 1) * P], xl[:])

        w_r = wp.tile([P, FT, D], f32r)
        wl = scr.tile([P, FT, D], f32, tag="load_scr")
        nc.sync.dma_start(wl[:], w_d[:].rearrange("(o p) d -> p o d", p=P))
        nc.vector.tensor_copy(w_r[:], wl[:])

        # mask accumulators; the per-row-tile |x| reductions are emitted
        # inside nj0's mm1 phase (see below) to keep ACT free early on.
        sumabs = const.tile([P, NT], f32)
        mask_sb = const.tile([P, NT], f32)

        cb = 0  # copyback counter for DVE/ACT alternation

        def copyback(dst, src, eng=None):
            nonlocal cb
            if eng is None:
                eng = "v" if cb % 2 == 0 else "s"
                cb += 1
            if eng == "v":
                nc.vector.tensor_copy(dst, src)
            else:
                nc.scalar.copy(dst, src)

        for nj in range(NJ):
            if nj == 0:
                a_strips = first_strips
            else:
                a_strips = []
                for j in range(NSUB):
                    ast = a_pool.tile([P, N], f32, tag="a_strip")
                    ni = nj * NSUB + j
                    nc.sync.dma_start(ast[:], a_d[ni * P : (ni + 1) * P, :])
                    a_strips.append(ast)

            at_sb = atp.tile([P, NT, NCHUNK], f32r, tag="at")
            tt_sb = ttp.tile([P, FT, NCHUNK], f32r, tag="tt")

            if nj == 0:
                # j-outer: transposes track the strip DMAs. Dummy matmuls
                # every other quad keep the HAM clock-gate open.
                for j in range(NSUB):
                    src = a_strips[j][:]
                    for q in range(NT // 4):
                        ps = ps_tp.tile([P, NCHUNK], f32, tag="pst")
                        for k in range(4):
                            mi = q * 4 + k
                            nc.tensor.transpose(
                                ps[:, k * P : (k + 1) * P],
                                src[:, mi * P : (mi + 1) * P],
                                ident[:],
                            )
                        dst = at_sb[:, q * 4 : (q + 1) * 4, j * P : (j + 1) * P]
                        copyback(dst, ps[:].rearrange("p (q f) -> p q f", q=4))
                        if q % 2 == 1:
                            warm_mm()
                # mm1, fi-outer: accumulation fi needs x column-chunk fi only.
                # The mask |x| reductions ride along, 4 per fi, so ACT takes
                # them where it has slack.
                for fi in range(FT):
                    pt = ps_mm.tile([P, NCHUNK], f32, tag="psm")
                    for mi in range(NT):
                        nc.tensor.matmul(
                            pt[:],
                            lhsT=x_r[:, mi, fi * P : (fi + 1) * P],
                            rhs=at_sb[:, mi],
                            start=(mi == 0),
                            stop=(mi == NT - 1),
                        )
                    for ni in range(fi * 4, fi * 4 + 4):
                        abs_scr = scr.tile([P, F], f32, tag="abs_scr")
                        nc.scalar.activation(
                            abs_scr[:],
                            x_r[:, ni],
                            AF.Abs,
                            accum_out=sumabs[:, ni : ni + 1],
                        )
                    copyback(tt_sb[:, fi], pt[:], eng="v" if fi % 2 == 0 else "s")
                nc.vector.tensor_scalar(
                    mask_sb[:], sumabs[:], 0.0, None, mybir.AluOpType.is_gt
                )
            else:
                # mi-outer with mm1 pipelined one m-tile behind the
                # transposes: regular matmuls interleave with transposes, so
                # the HAM stays warm and copybacks hide behind PE work.
                pt = [
                    ps_mm.tile([P, NCHUNK], f32, tag="psm", name=f"pt_{nj}_{fi}")
                    for fi in range(FT)
                ]

                def mm1_step(mi):
                    for fi in range(FT):
                        nc.tensor.matmul(
                            pt[fi][:],
                            lhsT=x_r[:, mi, fi * P : (fi + 1) * P],
                            rhs=at_sb[:, mi],
                            start=(mi == 0),
                            stop=(mi == NT - 1),
                        )

                for mi in range(NT):
                    ps = ps_tp.tile([P, NCHUNK], f32, tag="pst")
                    for j in range(NSUB):
                        nc.tensor.transpose(
                            ps[:, j * P : (j + 1) * P],
                            a_strips[j][:, mi * P : (mi + 1) * P],
                            ident[:],
                        )
                    copyback(at_sb[:, mi], ps[:])
                    if mi >= 1:
                        mm1_step(mi - 1)
                mm1_step(NT - 1)
                # engine-pinned parallel copybacks so mm2 can start after the
                # first one lands
                for fi in range(FT):
                    copyback(tt_sb[:, fi], pt[fi][:], eng="v" if fi % 2 == 0 else "s")

            # out rows for this chunk: accumulate over the 4 f-tiles, then
            # fused relu+mask on ACT, then store. Two dummies fill the PE
            # while the first tt copybacks land.
            warm_mm()
            warm_mm()
            for ns in range(NSUB):
                po = ps_o.tile([P, D], f32, tag="pso")
                for fi in range(FT):
                    nc.tensor.matmul(
                        po[:],
                        lhsT=tt_sb[:, fi, ns * P : (ns + 1) * P],
                        rhs=w_r[:, fi],
                        start=(fi == 0),
                        stop=(fi == FT - 1),
                    )
                ni = nj * NSUB + ns
                ob = outp.tile([P, D], f32, tag="ob")
                nc.scalar.activation(
                    ob[:], po[:], AF.Relu, scale=mask_sb[:, ni : ni + 1]
                )
                nc.sync.dma_start(o_d[ni * P : (ni + 1) * P, :], ob[:])

    nc.compile()
    return nc


def get_nc():
    if "nc" not in _CACHE:
        _CACHE["nc"] = _build_nc()
    return _CACHE["nc"]


def kernel(**inputs) -> np.ndarray:
    from concourse.bass_utils import run_bass_kernel_spmd

    x = np.ascontiguousarray(np.asarray(inputs["x"], dtype=np.float32))
    a = np.ascontiguousarray(np.asarray(inputs["a"], dtype=np.float32))
    w = np.ascontiguousarray(np.asarray(inputs["kernel"], dtype=np.float32))
    assert x.shape == (B, N, F) and a.shape == (B, N, N) and w.shape == (F, D)

    nc = get_nc()
    in_maps = [{"a": a[b], "x": x[b], "kernel": w} for b in range(B)]
    res = run_bass_kernel_spmd(nc, in_maps, core_ids=list(range(B)))
    return np.stack([res.results[b]["out"] for b in range(B)], axis=0)


# revision 8
# speedup vs baseline: 1.2177x; 1.1307x over previous
# GCN layer kernel for Trainium2: out[b] = relu((a[b] @ x[b]) @ W) * mask[b]
#
# Sharding: data-parallel over the batch (graph) dim. B=8 graphs, 8 cores,
# one graph per core; W replicated. Inputs are the FULL tensors; shards are
# sliced host-side and the per-core outputs stacked back together.
#
# Per-core dataflow (a: [2048,2048], x: [2048,512], W: [512,512]):
#   - a must be contracted over its column index; TensorE contracts over the
#     partition (row) index of both operands, so a is transposed on-chip via
#     PE transpose (fp32 has no DMA-transpose path), 128x128 tiles.
#   - t^T[f,n] = sum_m x[m,f] * aT[m,n]:  lhsT = x (as stored), rhs = aT.
#   - out[n,d] = sum_f t^T[f,n] * W[f,d]: lhsT = t^T, rhs = W (as stored),
#     which lands out in [n,d] layout for a direct DMA store.
#   - Matmuls run as float32r (full-rate fp32 mode; fp32 proper is 4x slower).
#     walrus requires f32r matmul operands to be produced by instructions that
#     round to f32r, so every operand tile is written by a DVE/ACT copy with
#     float32r output dtype (DMA-fed x/W get a one-time rounding copy).
#     Transposes stay fp32 (their a-strip/identity inputs are not rounded);
#     the PSUM->SBUF copyback CAST does the f32r rounding.
#   - mask[n] = any(x[n,:] != 0), computed as sum(|x[n,:]|) > 0, and applied
#     fused into the ReLU: relu(mask * t) == mask * relu(t) since mask >= 0.
#
# Schedule notes (from NTFF traces):
#   - The PE HAM clock-gate only counts REGULAR matmuls as activity;
#     transpose-mode matmuls run on the gated clock but do not un-throttle
#     it. A warm-up burst of fp32 identity matmuls (overlapping the initial
#     DMA wait) plus dummy matmuls inside transpose-only stretches and at
#     chunk boundaries keep the PE at K=8/8 (2.4 GHz).
#   - a is loaded as HALF-strips [128,1024] in a 10-slot pool so the next
#     chunk's strips prefetch while the current chunk computes; output
#     stores go through the GpSimd DMA queue so the Sync queue (loads)
#     never blocks behind the ReLU->store dependency chain.
#   - nj0 transposes are grouped per a-strip (j-outer) to start as soon as
#     the first half-strip lands. nj>=1 run mi-outer with mm1
#     software-pipelined one m-tile behind the transposes, so regular
#     matmuls interleave with transposes.
#   - The 16 mask |x| reductions are spread through nj0's mm1 phase so they
#     don't clog ACT ahead of the transpose copybacks.
#   - PSUM: 2 transpose + 4 mm1 + 2 out banks; warm-up/dummy matmuls borrow
#     the out-pool slots (idle at those points).

import numpy as np

B, N, F, D = 8, 2048, 512, 512
P = 128
NT = N // P        # 16 row-tiles of n (and of m, since a is square)
FT = F // P        # 4 tiles of f
NCHUNK = 512       # n is processed in chunks of 512 columns
NJ = N // NCHUNK   # 4
NSUB = NCHUNK // P # 4
HALF = N // 2      # a-strips are loaded in two 1024-column halves

_CACHE = {}


def _build_nc():
    from contextlib import ExitStack

    from concourse import bacc, mybir, tile
    from concourse.masks import make_identity

    f32 = mybir.dt.float32
    f32r = mybir.dt.float32r
    AF = mybir.ActivationFunctionType

    nc = bacc.Bacc(None)
    a_d = nc.dram_tensor("a", [N, N], f32, kind="ExternalInput")
    x_d = nc.dram_tensor("x", [N, F], f32, kind="ExternalInput")
    w_d = nc.dram_tensor("kernel", [F, D], f32, kind="ExternalInput")
    o_d = nc.dram_tensor("out", [N, D], f32, kind="ExternalOutput")

    with tile.TileContext(nc) as tc, ExitStack() as ctx:
        const = ctx.enter_context(tc.tile_pool(name="const", bufs=1))
        xp = ctx.enter_context(tc.tile_pool(name="xp", bufs=1))
        wp = ctx.enter_context(tc.tile_pool(name="wp", bufs=1))
        a_pool = ctx.enter_context(tc.tile_pool(name="a_pool", bufs=10))
        atp = ctx.enter_context(tc.tile_pool(name="atp", bufs=2))
        ttp = ctx.enter_context(tc.tile_pool(name="ttp", bufs=2))
        outp = ctx.enter_context(tc.tile_pool(name="outp", bufs=3))
        scr = ctx.enter_context(tc.tile_pool(name="scr", bufs=2))
        ps_tp = ctx.enter_context(tc.tile_pool(name="ps_tp", bufs=2, space="PSUM"))
        ps_mm = ctx.enter_context(tc.tile_pool(name="ps_mm", bufs=4, space="PSUM"))
        ps_o = ctx.enter_context(tc.tile_pool(name="ps_o", bufs=2, space="PSUM"))

        ident = const.tile([P, P], f32)
        make_identity(nc, ident[:])

        def warm_mm():
            # fp32 identity matmul: registers as HAM activity, output unused.
            # Borrows an out-pool PSUM slot (idle during transpose stretches).
            pw = ps_o.tile([P, D], f32, tag="pso", name="pw")
            nc.tensor.matmul(
                pw[:, :P], lhsT=ident[:], rhs=ident[:], start=True, stop=True
            )

        # HAM warm-up overlapping the initial DMA wait (>3.4us of cold-clock
        # PE activity flips the clock gate to 2.4 GHz before real work).
        for wu in range(10):
            warm_mm()

        def load_half_strips(nj):
            # a[nj*512:(nj+1)*512, :] as 4 row-strips x 2 column-halves.
            # h=0 halves first: transposes need them before h=1.
            halves = [[None, None] for _ in range(NSUB)]
            for h in range(2):
                for j in range(NSUB):
                    ah = a_pool.tile([P, HALF], f32, tag="a_half", name="ah")
                    ni = nj * NSUB + j
                    nc.sync.dma_start(
                        ah[:],
                        a_d[ni * P : (ni + 1) * P, h * HALF : (h + 1) * HALF],
                    )
                    halves[j][h] = ah
            return halves

        def strip_col(halves, j, mi):
            # columns mi*128:(mi+1)*128 of logical strip j
            h, o = divmod(mi, NT // 2)
            return halves[j][h][:, o * P : (o + 1) * P]

        first_halves = load_half_strips(0)

        # x: DMA fp32 column-chunks into scratch, round to f32r resident tile
        # [p, 16, 512] (m on partitions). mm1's fi-th accumulation needs only
        # column-chunk fi, so mm1 can ramp while later chunks stream in.
        x_r = xp.tile([P, NT, F], f32r)
        for c in range(4):
            xl = scr.tile([P, NT, P], f32, tag="load_scr")
            nc.sync.dma_start(
                xl[:], x_d[:, c * P : (c + 1) * P].rearrange("(o p) f -> p o f", p=P)
            )
            nc.vector.tensor_copy(x_r[:, :, c * P : (c + 1) * P], xl[:])

        w_r = wp.tile([P, FT, D], f32r)
        wl = scr.tile([P, FT, D], f32, tag="load_scr")
        nc.sync.dma_start(wl[:], w_d[:].rearrange("(o p) d -> p o d", p=P))
        nc.vector.tensor_copy(w_r[:], wl[:])

        # mask accumulators; the per-row-tile |x| reductions are emitted
        # inside nj0's mm1 phase (see below) to keep ACT free early on.
        sumabs = const.tile([P, NT], f32)
        mask_sb = const.tile([P, NT], f32)

        cb = 0  # copyback counter for DVE/ACT alternation

        def copyback(dst, src, eng=None):
            nonlocal cb
            if eng is None:
                eng = "v" if cb % 2 == 0 else "s"
                cb += 1
            if eng == "v":
                nc.vector.tensor_copy(dst, src)
            else:
                nc.scalar.copy(dst, src)

        halves = first_halves
        for nj in range(NJ):
            next_halves = load_half_strips(nj + 1) if nj + 1 < NJ else None

            at_sb = atp.tile([P, NT, NCHUNK], f32r, tag="at")
            tt_sb = ttp.tile([P, FT, NCHUNK], f32r, tag="tt")

            if nj == 0:
                # j-outer: transposes track the strip DMAs. Dummy matmuls
                # every other quad keep the HAM clock-gate open.
                for j in range(NSUB):
                    for q in range(NT // 4):
                        ps = ps_tp.tile([P, NCHUNK], f32, tag="pst")
                        for k in range(4):
                            mi = q * 4 + k
                            nc.tensor.transpose(
                                ps[:, k * P : (k + 1) * P],
                                strip_col(halves, j, mi),
                                ident[:],
                            )
                        dst = at_sb[:, q * 4 : (q + 1) * 4, j * P : (j + 1) * P]
                        copyback(dst, ps[:].rearrange("p (q f) -> p q f", q=4))
                        if q % 2 == 1:
                            warm_mm()
                # mm1, fi-outer: accumulation fi needs x column-chunk fi only.
                # The mask |x| reductions ride along, 4 per fi, so ACT takes
                # them where it has slack.
                for fi in range(FT):
                    pt = ps_mm.tile([P, NCHUNK], f32, tag="psm")
                    for mi in range(NT):
                        nc.tensor.matmul(
                            pt[:],
                            lhsT=x_r[:, mi, fi * P : (fi + 1) * P],
                            rhs=at_sb[:, mi],
                            start=(mi == 0),
                            stop=(mi == NT - 1),
                        )
                    for ni in range(fi * 4, fi * 4 + 4):
                        abs_scr = scr.tile([P, F], f32, tag="abs_scr")
                        nc.scalar.activation(
                            abs_scr[:],
                            x_r[:, ni],
                            AF.Abs,
                            accum_out=sumabs[:, ni : ni + 1],
                        )
                    copyback(tt_sb[:, fi], pt[:], eng="v" if fi % 2 == 0 else "s")
                nc.vector.tensor_scalar(
                    mask_sb[:], sumabs[:], 0.0, None, mybir.AluOpType.is_gt
                )
            else:
                # mi-outer with mm1 pipelined one m-tile behind the
                # transposes: regular matmuls interleave with transposes, so
                # the HAM stays warm and copybacks hide behind PE work.
                pt = [
                    ps_mm.tile([P, NCHUNK], f32, tag="psm", name=f"pt_{nj}_{fi}")
                    for fi in range(FT)
                ]

                def mm1_step(mi):
                    for fi in range(FT):
                        nc.tensor.matmul(
                            pt[fi][:],
                            lhsT=x_r[:, mi, fi * P : (fi + 1) * P],
                            rhs=at_sb[:, mi],
                            start=(mi == 0),
                            stop=(mi == NT - 1),
                        )

                for mi in range(NT):
                    ps = ps_tp.tile([P, NCHUNK], f32, tag="pst")
                    for j in range(NSUB):
                        nc.tensor.transpose(
                            ps[:, j * P : (j + 1) * P],
                            strip_col(halves, j, mi),
                            ident[:],
                        )
                    copyback(at_sb[:, mi], ps[:])
                    if mi >= 1:
                        mm1_step(mi - 1)
                mm1_step(NT - 1)
                # engine-pinned parallel copybacks so mm2 can start after the
                # first one lands
                for fi in range(FT):
                    copyback(tt_sb[:, fi], pt[fi][:], eng="v" if fi % 2 == 0 else "s")

            # out rows for this chunk: accumulate over the 4 f-tiles, then
            # fused relu+mask on ACT, then store (GpSimd DMA queue so loads
            # on Sync are never blocked). Two dummies fill the PE while the
            # first tt copybacks land.
            warm_mm()
            warm_mm()
            for ns in range(NSUB):
                po = ps_o.tile([P, D], f32, tag="pso")
                for fi in range(FT):
                    nc.tensor.matmul(
                        po[:],
                        lhsT=tt_sb[:, fi, ns * P : (ns + 1) * P],
                        rhs=w_r[:, fi],
                        start=(fi == 0),
                        stop=(fi == FT - 1),
                    )
                ni = nj * NSUB + ns
                ob = outp.tile([P, D], f32, tag="ob")
                nc.scalar.activation(
                    ob[:], po[:], AF.Relu, scale=mask_sb[:, ni : ni + 1]
                )
                nc.gpsimd.dma_start(o_d[ni * P : (ni + 1) * P, :], ob[:])

            halves = next_halves

    nc.compile()
    return nc


def get_nc():
    if "nc" not in _CACHE:
        _CACHE["nc"] = _build_nc()
    return _CACHE["nc"]


def kernel(**inputs) -> np.ndarray:
    from concourse.bass_utils import run_bass_kernel_spmd

    x = np.ascontiguousarray(np.asarray(inputs["x"], dtype=np.float32))
    a = np.ascontiguousarray(np.asarray(inputs["a"], dtype=np.float32))
    w = np.ascontiguousarray(np.asarray(inputs["kernel"], dtype=np.float32))
    assert x.shape == (B, N, F) and a.shape == (B, N, N) and w.shape == (F, D)

    nc = get_nc()
    in_maps = [{"a": a[b], "x": x[b], "kernel": w} for b in range(B)]
    res = run_bass_kernel_spmd(nc, in_maps, core_ids=list(range(B)))
    return np.stack([res.results[b]["out"] for b in range(B)], axis=0)


# revision 10
# speedup vs baseline: 1.2328x; 1.0124x over previous
# GCN layer kernel for Trainium2: out[b] = relu((a[b] @ x[b]) @ W) * mask[b]
#
# Sharding: data-parallel over the batch (graph) dim. B=8 graphs, 8 cores,
# one graph per core; W replicated. Inputs are the FULL tensors; shards are
# sliced host-side and the per-core outputs stacked back together.
#
# Per-core dataflow (a: [2048,2048], x: [2048,512], W: [512,512]):
#   - a must be contracted over its column index; TensorE contracts over the
#     partition (row) index of both operands, so a is transposed on-chip via
#     PE transpose (fp32 has no DMA-transpose path), 128x128 tiles.
#   - t^T[f,n] = sum_m x[m,f] * aT[m,n]:  lhsT = x (as stored), rhs = aT.
#   - out[n,d] = sum_f t^T[f,n] * W[f,d]: lhsT = t^T, rhs = W (as stored),
#     which lands out in [n,d] layout for a direct DMA store.
#   - Matmuls run as float32r (full-rate fp32 mode; fp32 proper is 4x slower).
#     walrus requires f32r matmul operands to be produced by instructions that
#     round to f32r, so every operand tile is written by a DVE/ACT copy with
#     float32r output dtype (DMA-fed x/W get a one-time rounding copy).
#     Transposes stay fp32 (their a-strip/identity inputs are not rounded);
#     the PSUM->SBUF copyback CAST does the f32r rounding.
#   - mask[n] = any(x[n,:] != 0), computed as sum(|x[n,:]|) > 0, and applied
#     fused into the ReLU: relu(mask * t) == mask * relu(t) since mask >= 0.
#
# Schedule notes (from NTFF traces):
#   - The PE HAM clock-gate only counts REGULAR matmuls as activity;
#     transpose-mode matmuls run on the gated clock but do not un-throttle
#     it. A warm-up burst of fp32 identity matmuls (overlapping the initial
#     DMA wait) plus dummy matmuls inside transpose-only stretches and at
#     chunk boundaries keep the PE at K=8/8 (2.4 GHz).
#   - a is loaded as HALF-strips [128,1024] in a 10-slot pool so the next
#     chunk's strips prefetch while the current chunk computes; output
#     stores go through the GpSimd DMA queue so the Sync queue (loads)
#     never blocks behind the ReLU->store dependency chain.
#   - nj0 transposes are grouped per a-strip (j-outer) to start as soon as
#     the first half-strip lands. nj>=1 run mi-outer with mm1
#     software-pipelined one m-tile behind the transposes, so regular
#     matmuls interleave with transposes.
#   - The 16 mask |x| reductions are spread through nj0's mm1 phase so they
#     don't clog ACT ahead of the transpose copybacks.
#   - PSUM: 2 transpose + 4 mm1 + 2 out banks; warm-up/dummy matmuls borrow
#     the out-pool slots (idle at those points).

import numpy as np

B, N, F, D = 8, 2048, 512, 512
P = 128
NT = N // P        # 16 row-tiles of n (and of m, since a is square)
FT = F // P        # 4 tiles of f
NCHUNK = 512       # n is processed in chunks of 512 columns
NJ = N // NCHUNK   # 4
NSUB = NCHUNK // P # 4
HALF = N // 2      # a-strips are loaded in two 1024-column halves

_CACHE = {}


def _build_nc():
    from contextlib import ExitStack

    from concourse import bacc, mybir, tile
    from concourse.masks import make_identity

    f32 = mybir.dt.float32
    f32r = mybir.dt.float32r
    AF = mybir.ActivationFunctionType

    nc = bacc.Bacc(None)
    a_d = nc.dram_tensor("a", [N, N], f32, kind="ExternalInput")
    x_d = nc.dram_tensor("x", [N, F], f32, kind="ExternalInput")
    w_d = nc.dram_tensor("kernel", [F, D], f32, kind="ExternalInput")
    o_d = nc.dram_tensor("out", [N, D], f32, kind="ExternalOutput")

    with tile.TileContext(nc) as tc, ExitStack() as ctx:
        const = ctx.enter_context(tc.tile_pool(name="const", bufs=1))
        xp = ctx.enter_context(tc.tile_pool(name="xp", bufs=1))
        wp = ctx.enter_context(tc.tile_pool(name="wp", bufs=1))
        a_pool = ctx.enter_context(tc.tile_pool(name="a_pool", bufs=10))
        atp = ctx.enter_context(tc.tile_pool(name="atp", bufs=2))
        ttp = ctx.enter_context(tc.tile_pool(name="ttp", bufs=2))
        outp = ctx.enter_context(tc.tile_pool(name="outp", bufs=3))
        scr = ctx.enter_context(tc.tile_pool(name="scr", bufs=2))
        ps_tp = ctx.enter_context(tc.tile_pool(name="ps_tp", bufs=2, space="PSUM"))
        ps_mm = ctx.enter_context(tc.tile_pool(name="ps_mm", bufs=4, space="PSUM"))
        ps_o = ctx.enter_context(tc.tile_pool(name="ps_o", bufs=2, space="PSUM"))

        ident = const.tile([P, P], f32)
        make_identity(nc, ident[:])

        def warm_mm():
            # fp32 identity matmul: registers as HAM activity, output unused.
            # Borrows an out-pool PSUM slot (idle during transpose stretches).
            pw = ps_o.tile([P, D], f32, tag="pso", name="pw")
            nc.tensor.matmul(
                pw[:, :P], lhsT=ident[:], rhs=ident[:], start=True, stop=True
            )

        # HAM warm-up overlapping the initial DMA wait (>3.4us of cold-clock
        # PE activity flips the clock gate to 2.4 GHz before real work).
        for wu in range(10):
            warm_mm()

        def load_half_strips(nj, h_range=(0, 1)):
            # a[nj*512:(nj+1)*512, :] as 4 row-strips x 2 column-halves.
            # h=0 halves first: transposes need them before h=1.
            halves = [[None, None] for _ in range(NSUB)]
            for h in h_range:
                for j in range(NSUB):
                    ah = a_pool.tile([P, HALF], f32, tag="a_half", name="ah")
                    ni = nj * NSUB + j
                    nc.sync.dma_start(
                        ah[:],
                        a_d[ni * P : (ni + 1) * P, h * HALF : (h + 1) * HALF],
                    )
                    halves[j][h] = ah
            return halves

        def strip_col(halves, j, mi):
            # columns mi*128:(mi+1)*128 of logical strip j
            h, o = divmod(mi, NT // 2)
            return halves[j][h][:, o * P : (o + 1) * P]

        # x: DMA fp32 column-chunks into scratch, round to f32r resident tile
        # [p, 16, 512] (m on partitions). mm1's fi-th accumulation needs only
        # column-chunk fi. Chunk 0 is interleaved between the two half-strip
        # DMA sets of nj0 so mm1 has its first lhsT as soon as the transposes
        # drain.
        x_r = xp.tile([P, NT, F], f32r)

        def load_x_chunk(c):
            xl = scr.tile([P, NT, P], f32, tag="load_scr", name="xl")
            nc.sync.dma_start(
                xl[:], x_d[:, c * P : (c + 1) * P].rearrange("(o p) f -> p o f", p=P)
            )
            nc.vector.tensor_copy(x_r[:, :, c * P : (c + 1) * P], xl[:])

        first_halves = load_half_strips(0, h_range=(0,))
        load_x_chunk(0)
        fh2 = load_half_strips(0, h_range=(1,))
        for j in range(NSUB):
            first_halves[j][1] = fh2[j][1]
        for c in range(1, 4):
            load_x_chunk(c)

        w_r = wp.tile([P, FT, D], f32r)
        wl = scr.tile([P, FT, D], f32, tag="load_scr")
        nc.sync.dma_start(wl[:], w_d[:].rearrange("(o p) d -> p o d", p=P))
        nc.vector.tensor_copy(w_r[:], wl[:])

        # mask accumulators; the per-row-tile |x| reductions are emitted
        # inside nj0's mm1 phase (see below) to keep ACT free early on.
        sumabs = const.tile([P, NT], f32)
        mask_sb = const.tile([P, NT], f32)

        cb = 0  # copyback counter for DVE/ACT alternation

        def copyback(dst, src, eng=None):
            nonlocal cb
            if eng is None:
                eng = "v" if cb % 2 == 0 else "s"
                cb += 1
            if eng == "v":
                nc.vector.tensor_copy(dst, src)
            else:
                nc.scalar.copy(dst, src)

        halves = first_halves
        for nj in range(NJ):
            next_halves = load_half_strips(nj + 1) if nj + 1 < NJ else None

            at_sb = atp.tile([P, NT, NCHUNK], f32r, tag="at")
            tt_sb = ttp.tile([P, FT, NCHUNK], f32r, tag="tt")

            if nj == 0:
                # j-outer: transposes track the strip DMAs. Dummy matmuls
                # every other quad keep the HAM clock-gate open.
                for j in range(NSUB):
                    for q in range(NT // 4):
                        ps = ps_tp.tile([P, NCHUNK], f32, tag="pst")
                        for k in range(4):
                            mi = q * 4 + k
                            nc.tensor.transpose(
                                ps[:, k * P : (k + 1) * P],
                                strip_col(halves, j, mi),
                                ident[:],
                            )
                        dst = at_sb[:, q * 4 : (q + 1) * 4, j * P : (j + 1) * P]
                        copyback(dst, ps[:].rearrange("p (q f) -> p q f", q=4))
                        if q % 2 == 1:
                            warm_mm()
                # mm1, fi-outer: accumulation fi needs x column-chunk fi only.
                # The mask |x| reductions ride along, 4 per fi, so ACT takes
                # them where it has slack.
                for fi in range(FT):
                    pt = ps_mm.tile([P, NCHUNK], f32, tag="psm")
                    for mi in range(NT):
                        nc.tensor.matmul(
                            pt[:],
                            lhsT=x_r[:, mi, fi * P : (fi + 1) * P],
                            rhs=at_sb[:, mi],
                            start=(mi == 0),
                            stop=(mi == NT - 1),
                        )
                    for ni in range(fi * 4, fi * 4 + 4):
                        abs_scr = scr.tile([P, F], f32, tag="abs_scr")
                        nc.scalar.activation(
                            abs_scr[:],
                            x_r[:, ni],
                            AF.Abs,
                            accum_out=sumabs[:, ni : ni + 1],
                        )
                    copyback(tt_sb[:, fi], pt[:], eng="v" if fi % 2 == 0 else "s")
                nc.vector.tensor_scalar(
                    mask_sb[:], sumabs[:], 0.0, None, mybir.AluOpType.is_gt
                )
            else:
                # mi-outer with mm1 pipelined one m-tile behind the
                # transposes: regular matmuls interleave with transposes, so
                # the HAM stays warm and copybacks hide behind PE work.
                pt = [
                    ps_mm.tile([P, NCHUNK], f32, tag="psm", name=f"pt_{nj}_{fi}")
                    for fi in range(FT)
                ]

                def mm1_step(mi):
                    for fi in range(FT):
                        nc.tensor.matmul(
                            pt[fi][:],
                            lhsT=x_r[:, mi, fi * P : (fi + 1) * P],
                            rhs=at_sb[:, mi],
                            start=(mi == 0),
                            stop=(mi == NT - 1),
                        )

                for mi in range(NT):
                    ps = ps_tp.tile([P, NCHUNK], f32, tag="pst")
                    for j in range(NSUB):
                        nc.tensor.transpose(
                            ps[:, j * P : (j + 1) * P],
                            strip_col(halves, j, mi),
                            ident[:],
                        )
                    # first copybacks pinned to DVE: ACT is still busy with
                    # the previous chunk's ReLUs at this point
                    copyback(at_sb[:, mi], ps[:], eng="v" if mi < 2 else None)
                    if mi >= 1:
                        mm1_step(mi - 1)
                mm1_step(NT - 1)
                # engine-pinned parallel copybacks so mm2 can start after the
                # first one lands
                for fi in range(FT):
                    copyback(tt_sb[:, fi], pt[fi][:], eng="v" if fi % 2 == 0 else "s")

            # out rows for this chunk: accumulate over the 4 f-tiles, then
            # fused relu+mask on ACT, then store (GpSimd DMA queue so loads
            # on Sync are never blocked). Two dummies fill the PE while the
            # first tt copybacks land.
            warm_mm()
            warm_mm()
            for ns in range(NSUB):
                po = ps_o.tile([P, D], f32, tag="pso")
                for fi in range(FT):
                    nc.tensor.matmul(
                        po[:],
                        lhsT=tt_sb[:, fi, ns * P : (ns + 1) * P],
                        rhs=w_r[:, fi],
                        start=(fi == 0),
                        stop=(fi == FT - 1),
                    )
                ni = nj * NSUB + ns
                ob = outp.tile([P, D], f32, tag="ob")
                nc.scalar.activation(
                    ob[:], po[:], AF.Relu, scale=mask_sb[:, ni : ni + 1]
                )
                nc.gpsimd.dma_start(o_d[ni * P : (ni + 1) * P, :], ob[:])

            halves = next_halves

    nc.compile()
    return nc


def get_nc():
    if "nc" not in _CACHE:
        _CACHE["nc"] = _build_nc()
    return _CACHE["nc"]


def kernel(**inputs) -> np.ndarray:
    from concourse.bass_utils import run_bass_kernel_spmd

    x = np.ascontiguousarray(np.asarray(inputs["x"], dtype=np.float32))
    a = np.ascontiguousarray(np.asarray(inputs["a"], dtype=np.float32))
    w = np.ascontiguousarray(np.asarray(inputs["kernel"], dtype=np.float32))
    assert x.shape == (B, N, F) and a.shape == (B, N, N) and w.shape == (F, D)

    nc = get_nc()
    in_maps = [{"a": a[b], "x": x[b], "kernel": w} for b in range(B)]
    res = run_bass_kernel_spmd(nc, in_maps, core_ids=list(range(B)))
    return np.stack([res.results[b]["out"] for b in range(B)], axis=0)
